# revision 9
# baseline (speedup 1.0000x reference)
"""DGMG forward-loss Trainium2 kernel (Bass/Tile), data-parallel over 8 NeuronCores.

Model (B=128 graphs, N=32 nodes, D=G=256, T=2 GCN rounds): a 32-step sequential
graph-generation loop; each step runs small MLPs (add-node, init-node, add-edge,
select-node) and a 2-round GCN on a growing path graph, accumulating a scalar
loss of log-softmax / log-sigmoid terms.

Sharding: batch 128 -> 16 graphs per core (everything else replicated).
Per core, activations live transposed in SBUF: [features -> partitions,
node*16 + batch -> free], bf16.

v2 rewrite (chain-focused):
- select-node (fs) work for step v is emitted during step v+1 (fs2 during
  v+2) so it never stalls the per-step critical chain; 2 state buffers + tmp.
- fs1 hv-part computed once per step (8 tiny matmuls into psum, copied to
  SBUF), then spread across candidates by an identity-weight matmul with a
  0-stride rhs: halves the fs1 streaming vs running it through fs_W1.
- GCN neighbor-mix Z in 2 DVE tensor_tensor ops (pair-sum A=u_l+u_r;
  Z=A+u_mid); path endpoints (deg-2 nodes) on GpSimd in parallel; relu+bias
  split DVE/GpSimd (ACT only does sigmoids).
- biases folded via scalar_tensor_tensor (psum + per-partition bias bcast)
  or tiny const-rhs matmuls at psum-group start; finit sigmoid is one ACT op.
- all weights packed into one dram tensor, DMA'd in first-needed-first column
  chunks; the 1MB shifted fs2 weight is built on device (memset + strided
  sbuf->sbuf DMA of 32KB).
- tail exp/ln clustered to avoid ACT table thrash.
"""
import sys

for _p in ('/opt/trn_rl_repo/concourse', '/opt/trn_rl_repo'):
    if _p not in sys.path:
        sys.path.insert(0, _p)

import numpy as np
import ml_dtypes

bf16 = ml_dtypes.bfloat16

# ----------------------------------------------------------------------------
# compat: this container's walrus accepts only ONE sem-wait / sem-update per
# instruction; split extras onto adjacent NOPs. Also register the NTFF profile
# hook that bass_utils expects under axon (module missing from the image).
# ----------------------------------------------------------------------------

def _install_axon_hook():
    import types
    if 'antenv.axon_hooks' in sys.modules:
        return
    import antenv
    mod = types.ModuleType('antenv.axon_hooks')
    _hook = [None]
    mod.set_axon_ntff_profile_hook = lambda h: _hook.__setitem__(0, h)
    mod.get_axon_ntff_profile_hook = lambda: _hook[0]
    sys.modules['antenv.axon_hooks'] = mod
    antenv.axon_hooks = mod
    try:
        from trn_agent_boot.trn_boot import _ntff_profile_via_ctypes
        mod.set_axon_ntff_profile_hook(
            _ntff_profile_via_ctypes('/opt/axon/libaxon_pjrt.so'))
    except Exception:
        pass


def _split_multiwait(nc):
    import concourse.mybir as mybir
    for fn in nc.m.functions:
        for bb in fn.blocks:
            out, changed = [], False
            for inst in bb.instructions:
                si = inst.sync_info
                if si is None or (len(si.on_wait) <= 1 and len(si.on_update) <= 1):
                    out.append(inst)
                    continue
                changed = True
                waits, updates = list(si.on_wait), list(si.on_update)
                for w in waits[:-1]:
                    out.append(mybir.InstNoOp(
                        name=f"{inst.name}_w{len(out)}", ins=[], outs=[],
                        engine=inst.engine,
                        sync_info=mybir.SyncInfo(on_wait=[w], on_update=[]),
                        bass_nofuse=True))
                inst.sync_info = mybir.SyncInfo(
                    on_wait=waits[-1:], on_update=updates[:1])
                out.append(inst)
                for i, u in enumerate(updates[1:]):
                    out.append(mybir.InstNoOp(
                        name=f"{inst.name}_u{i}", ins=[], outs=[],
                        engine=inst.engine,
                        sync_info=mybir.SyncInfo(on_wait=[], on_update=[u]),
                        bass_nofuse=True))
            if changed:
                bb.instructions = out


# ----------------------------------------------------------------------------
# problem constants (hardcoded per the harness contract)
# ----------------------------------------------------------------------------
D = 256
G = 256
T = 2
B = 128
NN = 32           # nodes per graph
NC = 8            # cores
BL = B // NC      # batch per core = 16

_BUILD_CACHE = {}

# wpack column layout (bf16), in DMA order (first-needed first)
_WOFF = {}
_wc = 0
for _name, _w in [("wfi1", 512), ("wfi2", 512), ("ident2", 32), ("bfi1T", 128),
                  ("wgcn0", 512), ("wgcn1", 512), ("wg3", 512),
                  ("wfs1", 2048), ("ident4", 64), ("bfs1T", 128),
                  ("wfs2rep", 128), ("e128", 128),
                  ("wfan1", 512), ("wfae1", 2048),
                  ("wfan2d", 2), ("wfae2p", 4)]:
    _WOFF[_name] = _wc
    _wc += _w
_WTOT = _wc
# DMA chunk boundaries (columns)
_WCHUNK1 = _WOFF["wgcn0"]   # finit weights
_WCHUNK2 = _WOFF["wfan1"]   # loop weights
_WSIZES = {"wfi1": 512, "wfi2": 512, "wgcn0": 512, "wgcn1": 512,
           "wg3": 512, "wfs1": 2048, "wfan1": 512, "wfae1": 2048,
           "wfan2d": 2, "wfae2p": 4}


def _build_program():
    """Build the per-core Bass program (same program on all 8 cores)."""
    import concourse.bass as bass
    import concourse.mybir as mybir
    from concourse.tile import TileContext

    F32, BF16 = mybir.dt.float32, mybir.dt.bfloat16
    AF = mybir.ActivationFunctionType
    ALU = mybir.AluOpType

    nc = bass.Bass()

    wpack = nc.dram_tensor("wpack", [128, _WTOT], BF16, kind="ExternalInput")
    bpack = nc.dram_tensor("bpack", [128, 16], F32, kind="ExternalInput")
    bgsd = nc.dram_tensor("bgsd", [128, 64], F32, kind="ExternalInput")
    mpack = nc.dram_tensor("mpack", [32, 1024], F32, kind="ExternalInput")
    hg0 = nc.dram_tensor("hg0", [128, 32], BF16, kind="ExternalInput")

    o_fan = nc.dram_tensor("o_fan", [1, 544], F32, kind="ExternalOutput")
    o_fae1 = nc.dram_tensor("o_fae1", [1, 512], F32, kind="ExternalOutput")
    o_fae2 = nc.dram_tensor("o_fae2", [1, 512], F32, kind="ExternalOutput")
    o_fs_lse = nc.dram_tensor("o_fs_lse", [32, 16], F32, kind="ExternalOutput")
    o_fs_src = nc.dram_tensor("o_fs_src", [32, 16], F32, kind="ExternalOutput")

    from contextlib import ExitStack
    with TileContext(nc) as tc, ExitStack() as ctx:
        wp = ctx.enter_context(tc.tile_pool(name="w", bufs=1))
        st = ctx.enter_context(tc.tile_pool(name="st", bufs=1))
        ps_gcn = ctx.enter_context(tc.tile_pool(name="ps_gcn", bufs=1, space="PSUM"))
        ps_fsq = ctx.enter_context(tc.tile_pool(name="ps_fsq", bufs=2, space="PSUM"))
        ps_sm = ctx.enter_context(tc.tile_pool(name="ps_sm", bufs=3, space="PSUM"))
        ps_sc = ctx.enter_context(tc.tile_pool(name="ps_sc", bufs=1, space="PSUM"))

        # ---------------- SBUF tiles ----------------
        wpk = wp.tile([128, _WTOT], BF16, name="wpk", tag="wpk")
        bpk = wp.tile([128, 16], F32, name="bpk", tag="bpk")
        bgs = wp.tile([128, 2, 32], F32, name="bgs", tag="bgs")
        mpk = wp.tile([32, 1024], F32, name="mpk", tag="mpk")
        thg0 = wp.tile([128, 32], BF16, name="thg0", tag="thg0")
        wfs2s = wp.tile([128, 4, 32, 32], BF16, name="wfs2s", tag="wfs2s")

        def W(name, kh=2):  # [128, kh, 128*mh...] view of wpack
            o = _WOFF[name]
            return wpk[:, o:o + _WSIZES[name]].rearrange("p (k m) -> p k m", k=kh)

        twfi1, twfi2 = W("wfi1"), W("wfi2")
        twgcn = [W("wgcn0"), W("wgcn1")]
        twg3 = W("wg3")
        twfs1 = W("wfs1", kh=4)
        twfan1 = W("wfan1")
        twfae1 = W("wfae1", kh=4)
        twfan2d = W("wfan2d")
        twfae2p = W("wfae2p", kh=4)
        tident2 = wpk[0:2, _WOFF["ident2"]:_WOFF["ident2"] + 32].rearrange(
            "p (k b) -> p k b", k=2)
        tident4 = wpk[0:4, _WOFF["ident4"]:_WOFF["ident4"] + 64].rearrange(
            "p (k b) -> p k b", k=4)
        tbfi1T = wpk[0:2, _WOFF["bfi1T"]:_WOFF["bfi1T"] + 128]
        tbfs1T = wpk[0:4, _WOFF["bfs1T"]:_WOFF["bfs1T"] + 128]
        twfs2rep = wpk[:, _WOFF["wfs2rep"]:_WOFF["wfs2rep"] + 128]
        te128 = wpk[:, _WOFF["e128"]:_WOFF["e128"] + 128]

        tbgcn = [bpk[:, 0:2], bpk[:, 2:4]]
        tbfi2third = bpk[:, 4:6]
        tbfan1 = bpk[:, 6:8]
        tbfae1 = bpk[:, 8:12]
        tfscal = bpk[0:1, 12:16]
        tmaskneg = mpk[:, 0:512]
        tonehot = mpk[:, 512:1024]
        thg = thg0[:].rearrange("p (k b) -> p k b", k=2)

        # state: state(v) lives in S[v % 2]; Tb holds the mid-GCN round.
        S = [st.tile([128, 2, 512], BF16, name=f"S{i}", tag=f"S{i}")
             for i in range(2)]
        Tb = st.tile([128, 2, 512], BF16, name="Tb", tag="Tb")
        uv_hist = st.tile([128, 2, 512], BF16, name="uv_hist", tag="uv_hist")
        hG_hist = st.tile([128, 2, 528], BF16, name="hG_hist", tag="hG_hist")
        Zt = st.tile([128, 2, 512], BF16, name="Zt", tag="Zt")
        At = st.tile([128, 2, 512], BF16, name="At", tag="At")
        s1fs = [st.tile([128, 4, 512], BF16, name=f"s1fs{i}", tag=f"s1fs{i}")
                for i in range(2)]
        s_hv = [st.tile([128, 4, 16], BF16, name=f"s_hv{i}", tag=f"s_hv{i}")
                for i in range(2)]
        s_hist = st.tile([32, 512], F32, name="s_hist", tag="s_hist")
        ps_score = ps_sc.tile([32, 512], F32, name="score", tag="score")

        # ---------------- input DMAs ----------------
        # SP queue: first-needed first; Pool queue: tail weights + bgs.
        nc.sync.dma_start(out=thg0[:], in_=hg0[:])
        nc.sync.dma_start(out=bpk[:], in_=bpack[:])
        nc.sync.dma_start(out=wpk[:, 0:_WCHUNK1], in_=wpack[:, 0:_WCHUNK1])
        nc.sync.dma_start(out=wpk[:, _WCHUNK1:_WCHUNK2],
                          in_=wpack[:, _WCHUNK1:_WCHUNK2])
        nc.gpsimd.dma_start(out=bgs[:],
                            in_=bgsd[:].rearrange("p (k v) -> p k v", k=2))
        nc.gpsimd.dma_start(out=wpk[:, _WCHUNK2:_WTOT],
                            in_=wpack[:, _WCHUNK2:_WTOT])

        # s_hist zeros; shifted fs2 weights built on device:
        #   wfs2s[p, kh, v, m] = fs_W2[kh*128+p] * (m == v)
        nc.vector.memset(s_hist[:], 0.0)
        nc.vector.memset(wfs2s[:], 0.0)
        wfs2s_flat = wfs2s[:].rearrange("p k v m -> p k (v m)")
        for _kh in range(4):
            nc.sync.dma_start(out=wfs2s_flat[:, _kh, 0:1024:33],
                              in_=twfs2rep[:, _kh * 32:(_kh + 1) * 32])
        nc.sync.dma_start(out=mpk[:], in_=mpack[:])

        # hG_hist[0] = hG_hist[1] = hG0  (off-chain, for the fan/fae tails)
        nc.gpsimd.tensor_copy(hG_hist[:, :, 0:16], thg)
        nc.gpsimd.tensor_copy(hG_hist[:, :, 16:32], thg)

        # ---------------- emit helpers ----------------
        def finit_emit(v):
            """add-node init MLP for step v; writes u_v into state-in buf."""
            sin = S[(v - 1) % 2] if v >= 1 else S[0]
            rhs = thg if v == 0 else hG_hist[:, :, 16 * v:16 * v + 16]
            pa = ps_sm.tile([128, 2, 16], F32, tag="sm", name=f"pa{v}")
            nc.tensor.matmul(pa[:], tbfi1T, tident2,
                             start=True, stop=False, skip_group_check=True)
            for mh in range(2):
                for kh in range(2):
                    nc.tensor.matmul(
                        pa[:, mh, 0:16],
                        twfi1[:, kh, mh * 128:(mh + 1) * 128],
                        rhs[:, kh, :],
                        start=False, stop=(mh == 1 and kh == 1),
                        skip_group_check=True)
            s1fi = st.tile([128, 2, 16], BF16, tag="s1fi", name=f"s1fi{v}")
            nc.scalar.activation(s1fi[:], pa[:], AF.Sigmoid)
            pb = ps_sm.tile([128, 2, 16], F32, tag="sm", name=f"pb{v}")
            for mh in range(2):
                for kh in range(2):
                    nc.tensor.matmul(
                        pb[:, mh, 0:16],
                        twfi2[:, kh, mh * 128:(mh + 1) * 128],
                        s1fi[:, kh, :],
                        start=(kh == 0), stop=(kh == 1))
            # u_v = (pb + bfi2)/3  (DVE, on-chain)
            nc.vector.scalar_tensor_tensor(
                out=sin[:, :, 16 * v:16 * v + 16], in0=pb[:],
                scalar=1.0 / 3.0,
                in1=tbfi2third[:, :].unsqueeze(2).broadcast_to([128, 2, 16]),
                op0=ALU.mult, op1=ALU.add)
            # archive for the fs hv-part / fae tail (off-chain)
            nc.gpsimd.tensor_copy(uv_hist[:, :, 16 * v:16 * v + 16],
                                  sin[:, :, 16 * v:16 * v + 16])

        def pfhv_emit(v):
            """fs1 hv-part for step v: pfhv = bfs1 + W1b @ (3 u_v)."""
            pfhv = ps_sm.tile([128, 4, 16], F32, tag="sm", name=f"pfhv{v}")
            nc.tensor.matmul(pfhv[:], tbfs1T, tident4,
                             start=True, stop=False, skip_group_check=True)
            for mh in range(4):
                for kh in range(2):
                    nc.tensor.matmul(
                        pfhv[:, mh, :],
                        twfs1[:, 2 + kh, mh * 128:(mh + 1) * 128],
                        uv_hist[:, kh, 16 * v:16 * v + 16],
                        start=False, stop=(mh == 3 and kh == 1),
                        skip_group_check=True)
            return pfhv

        def fsq_emit(v, mh):
            """fs1 quarter mh for step v: cand matmuls + identity hv-spread."""
            w = 16 * v
            sprev = S[(v - 1) % 2]  # state(v-1)
            pf = ps_fsq.tile([128, 512], F32, tag="fsq", name=f"pf{v}_{mh}")
            for kh in range(2):
                nc.tensor.matmul(
                    pf[:, 0:w],
                    twfs1[:, kh, mh * 128:(mh + 1) * 128],
                    sprev[:, kh, 0:w],
                    start=(kh == 0), stop=False, skip_group_check=True)
            nc.tensor.matmul(
                pf[:, 0:w].rearrange("p (n b) -> p n b", b=16),
                te128,
                s_hv[v % 2][:, mh, :].unsqueeze(1).broadcast_to([128, v, 16]),
                start=False, stop=True, skip_group_check=True)
            return pf

        def fsq_sig(v, mh, pf):
            nc.scalar.activation(s1fs[v % 2][:, mh, 0:16 * v],
                                 pf[:, 0:16 * v], AF.Sigmoid)

        def fs2_emit(v):
            w = 16 * v
            for kh in range(4):
                nc.tensor.matmul(ps_score[:, 0:w], wfs2s[:, kh, v, :],
                                 s1fs[v % 2][:, kh, 0:w],
                                 start=(v == 1 and kh == 0),
                                 stop=(v == NN - 1 and kh == 3),
                                 skip_group_check=True)

        def zprep_emit(v, src):
            """Z (neighbor mix) for step v (c=v+1 nodes) from src buffer."""
            c = v + 1
            if c == 2:
                nc.gpsimd.tensor_add(Zt[:, :, 0:16], src[:, :, 0:16],
                                     src[:, :, 16:32])
                nc.gpsimd.tensor_scalar(out=Zt[:, :, 0:16], in0=Zt[:, :, 0:16],
                                        scalar1=1.5, scalar2=None, op0=ALU.mult)
                return
            # interior on DVE: A[1:c-1] = u[0:c-2] + u[2:c]; Z = A + u[1:c-1]
            nc.vector.tensor_add(
                At[:, :, 16:16 * (c - 1)], src[:, :, 0:16 * (c - 2)],
                src[:, :, 32:16 * c])
            nc.vector.tensor_add(
                Zt[:, :, 16:16 * (c - 1)], At[:, :, 16:16 * (c - 1)],
                src[:, :, 16:16 * (c - 1)])
            # path endpoints (deg-2 nodes) on Pool: Z_e = 1.5*(u_e + u_n)
            e0 = src[:, :, 0:16 * c].rearrange("p k (n b) -> p k n b", b=16)
            zb = Zt[:, :, 0:16 * c].rearrange("p k (n b) -> p k n b", b=16)
            if c == 3:
                nbr = e0[:, :, 1:2, :].broadcast_to([128, 2, 2, 16])
            else:
                nbr = e0[:, :, 1:c - 1:c - 3, :]
            nc.gpsimd.tensor_add(
                zb[:, :, 0:c:c - 1, :], e0[:, :, 0:c:c - 1, :], nbr)
            nc.gpsimd.tensor_scalar(
                out=zb[:, :, 0:c:c - 1, :], in0=zb[:, :, 0:c:c - 1, :],
                scalar1=1.5, scalar2=None, op0=ALU.mult)

        def gcn_mm_emit(v, t):
            c = v + 1
            pg = ps_gcn.tile([128, 2, 512], F32, tag="gcn", name=f"pg{v}_{t}")
            if c == 2:
                rhs = [Zt[:, kh, 0:16].unsqueeze(1).broadcast_to([128, 2, 16])
                       for kh in range(2)]
            else:
                rhs = [Zt[:, kh, 0:16 * c] for kh in range(2)]
            for mh in range(2):
                for kh in range(2):
                    nc.tensor.matmul(
                        pg[:, mh, 0:16 * c],
                        twgcn[t][:, kh, mh * 128:(mh + 1) * 128],
                        rhs[kh],
                        start=(kh == 0), stop=(kh == 1))
            return pg

        def relu_emit(v, t, pg, dst):
            c = v + 1
            nc.vector.tensor_scalar(
                out=dst[:, 0, 0:16 * c], in0=pg[:, 0, 0:16 * c],
                scalar1=tbgcn[t][:, 0:1], scalar2=0.0,
                op0=ALU.add, op1=ALU.max)
            nc.scalar.activation(
                dst[:, 1, 0:16 * c], pg[:, 1, 0:16 * c],
                AF.Relu, bias=tbgcn[t][:, 1:2])

        def readout_emit(v, newcur):
            c = v + 1
            pr = ps_sm.tile([128, 2, 16], F32, tag="sm", name=f"pr{v}")
            nc4 = newcur[:].rearrange("p k (n b) -> p k n b", b=16)
            for mh in range(2):
                out_bc = (pr[:, mh, 0:16].unsqueeze(1)
                          .broadcast_to([128, c, 16]))
                for kh in range(2):
                    nc.tensor.matmul(
                        out_bc,
                        twg3[:, kh, mh * 128:(mh + 1) * 128],
                        nc4[:, kh, 0:c, :],
                        start=(kh == 0), stop=(kh == 1))
            # hG(v+1) = pr + (v+1)*bg   (one DVE op, bf16 out)
            nc.vector.scalar_tensor_tensor(
                out=hG_hist[:, :, 16 * (v + 1):16 * (v + 2)], in0=pr[:],
                scalar=1.0,
                in1=bgs[:, :, v:v + 1].broadcast_to([128, 2, 16]),
                op0=ALU.mult, op1=ALU.add)

        # ---------------- main loop ----------------
        for v in range(NN):
            c = v + 1
            # --- finit (chain head) ---
            if v == 1:
                # hG(1) == hG(0) => hv_1 == hv_0 (reference quirk: no edge yet)
                nc.vector.tensor_copy(S[0][:, :, 16:32], S[0][:, :, 0:16])
                nc.gpsimd.tensor_copy(uv_hist[:, :, 16:32],
                                      uv_hist[:, :, 0:16])
            else:
                finit_emit(v)

            # --- deferred fs pieces (PE fillers, never stall the chain) ---
            if v >= 3:
                fs2_emit(v - 2)
            pf01 = [fsq_emit(v - 1, mh) for mh in (0, 1)] if v >= 2 else None

            if v >= 1:
                zprep_emit(v, S[(v - 1) % 2])
                pg0 = gcn_mm_emit(v, 0)
                relu_emit(v, 0, pg0, Tb)
                if pf01 is not None:
                    fsq_sig(v - 1, 0, pf01[0])
                    fsq_sig(v - 1, 1, pf01[1])
                pf23 = [fsq_emit(v - 1, mh) for mh in (2, 3)] if v >= 2 else None
                zprep_emit(v, Tb)
                pg1 = gcn_mm_emit(v, 1)
                newcur = S[v % 2]
                relu_emit(v, 1, pg1, newcur)
                if pf23 is not None:
                    fsq_sig(v - 1, 2, pf23[0])
                    fsq_sig(v - 1, 3, pf23[1])
                readout_emit(v, newcur)
                # hv-part of fs(v): tiny matmuls at PE queue end + DVE copy
                pfhv = pfhv_emit(v)
                nc.vector.tensor_copy(s_hv[v % 2][:], pfhv[:])

        # ---------------- post-loop: fs(31) quarters, fs2(30), fs2(31) ------
        fs2_emit(NN - 2)
        pfq = [fsq_emit(NN - 1, mh) for mh in range(4)]
        for mh in range(4):
            fsq_sig(NN - 1, mh, pfq[mh])
        fs2_emit(NN - 1)

        # ------------------------- deferred loss tails ----------------------
        s1fan = st.tile([128, 2, 528], BF16, name="s1fan", tag="s1fan")
        sp_fan = st.tile([1, 544], F32, name="sp_fan", tag="sp_fan")

        # add-node head: d = (l0 - l1), terms softplus(+-d)
        for (c0, cw) in ((0, 272), (272, 256)):
            pl = ps_gcn.tile([128, 2, 512], F32, name=f"pl{c0}", tag="gcn")
            for mh in range(2):
                for kh in range(2):
                    nc.tensor.matmul(
                        pl[:, mh, 0:cw],
                        twfan1[:, kh, mh * 128:(mh + 1) * 128],
                        hG_hist[:, kh, c0:c0 + cw],
                        start=(kh == 0), stop=(kh == 1))
            for mh in range(2):
                nc.scalar.activation(s1fan[:, mh, c0:c0 + cw],
                                     pl[:, mh, 0:cw], AF.Sigmoid,
                                     bias=tbfan1[:, mh:mh + 1])
        # add-edge layer-1 sigmoids (uv_hist carries u = hv/3; x3 folded into
        # the packed fae_W1 bottom-half rows)
        s1fae2 = st.tile([128, 4, 512], BF16, name="s1fae2", tag="s1fae2")
        for gi, (cols_g, cols_v, s1buf) in enumerate((
                ((16, 512), (16, 512), s1fs[0]),
                ((16, 528), (0, 512), s1fae2))):
            gw = cols_g[1] - cols_g[0]
            for half in range(2):
                pfa = ps_fsq.tile([128, 512], F32, tag="fsq",
                                  name=f"pfae{gi}_{half}a")
                pfb = ps_fsq.tile([128, 512], F32, tag="fsq",
                                  name=f"pfae{gi}_{half}b")
                for mh2 in range(2):
                    mh = half * 2 + mh2
                    dst = pfa if mh2 == 0 else pfb
                    for kh in range(4):
                        rhs = (hG_hist[:, kh, cols_g[0]:cols_g[1]] if kh < 2
                               else uv_hist[:, kh - 2, cols_v[0]:cols_v[1]])
                        nc.tensor.matmul(
                            dst[:, 0:gw],
                            twfae1[:, kh, mh * 128:(mh + 1) * 128],
                            rhs, start=(kh == 0), stop=(kh == 3))
                for mh2 in range(2):
                    mh = half * 2 + mh2
                    nc.scalar.activation(
                        s1buf[:, mh, 0:gw], (pfa if mh2 == 0 else pfb)[:, 0:gw],
                        AF.Sigmoid, bias=tbfae1[:, mh:mh + 1])

        # --- tail phase 2: all layer-2 matmuls (PE) + select-node DVE prep.
        pdA = ps_sm.tile([1, 512], F32, name="pdA", tag="sm")
        for kh in range(2):
            nc.tensor.matmul(pdA[0:1, 0:512], twfan2d[:, kh, :],
                             s1fan[:, kh, 0:512],
                             start=(kh == 0), stop=(kh == 1))
        pdB = ps_sm.tile([1, 512], F32, name="pdB", tag="sm")
        for kh in range(2):
            nc.tensor.matmul(pdB[0:1, 0:16], twfan2d[:, kh, :],
                             s1fan[:, kh, 512:528],
                             start=(kh == 0), stop=(kh == 1))
        pz1 = ps_sm.tile([1, 512], F32, name="pz1", tag="sm")
        for kh in range(4):
            nc.tensor.matmul(pz1[0:1, 0:496], twfae2p[:, kh, :],
                             s1fs[0][:, kh, 0:496],
                             start=(kh == 0), stop=(kh == 3))
        pz2 = ps_gcn.tile([128, 2, 512], F32, name="pz2", tag="gcn")
        for kh in range(4):
            nc.tensor.matmul(pz2[0:1, 0, 0:512], twfae2p[:, kh, :],
                             s1fae2[:, kh, 0:512],
                             start=(kh == 0), stop=(kh == 3))
        s_e = st.tile([32, 512], F32, name="s_e", tag="s_e")
        s_sum = st.tile([32, 16], F32, name="s_sum", tag="s_sum")
        s_lse = st.tile([32, 16], F32, name="s_lse", tag="s_lse")
        s_srcm = st.tile([32, 512], F32, name="s_srcm", tag="s_srcm")
        s_src = st.tile([32, 16], F32, name="s_src", tag="s_src")
        nc.vector.tensor_copy(s_hist[0:32, 0:496], ps_score[:, 0:496])
        nc.vector.tensor_add(s_hist[:], s_hist[:], tmaskneg[:])
        nc.vector.tensor_mul(s_srcm[:], s_hist[:], tonehot[:])
        nc.vector.reduce_sum(
            s_src[:], s_srcm[:].rearrange("p (n b) -> p b n", b=16),
            axis=mybir.AxisListType.X)
        nc.sync.dma_start(out=o_fs_src[:], in_=s_src[:])

        # --- tail phase 3: cluster Exp then Ln (2 table switches).
        e_fan = st.tile([1, 544], F32, name="e_fan", tag="e_fan")
        e_fae = st.tile([1, 512], F32, name="e_fae", tag="e_fae")
        e_fae2 = st.tile([1, 512], F32, name="e_fae2", tag="e_fae2")
        sp_fae = st.tile([1, 512], F32, name="sp_fae", tag="sp_fae")
        sp_fae2 = st.tile([1, 512], F32, name="sp_fae2", tag="sp_fae2")
        nc.scalar.activation(e_fan[0:1, 0:512], pdA[0:1, 0:512],
                             AF.Exp, bias=tfscal[0:1, 0:1])              # exp(d+db)
        nc.scalar.activation(e_fan[0:1, 512:528], pdB[0:1, 0:16],
                             AF.Exp, bias=tfscal[0:1, 1:2], scale=-1.0)  # exp(-d-db)
        nc.scalar.activation(e_fae[0:1, 0:496], pz1[0:1, 0:496],
                             AF.Exp, bias=tfscal[0:1, 3:4], scale=-1.0)  # exp(-z1-b2)
        nc.scalar.activation(e_fae2[0:1, 0:512], pz2[0:1, 0, 0:512],
                             AF.Exp, bias=tfscal[0:1, 2:3])              # exp(z2+b2)
        nc.scalar.activation(s_e[:], s_hist[:], AF.Exp)
        nc.vector.reduce_sum(
            s_sum[:], s_e[:].rearrange("p (n b) -> p b n", b=16),
            axis=mybir.AxisListType.X)
        nc.vector.memset(sp_fae[0:1, 496:512], 0.0)
        nc.scalar.activation(sp_fan[0:1, 0:528], e_fan[0:1, 0:528],
                             AF.Ln, bias=1.0)
        nc.sync.dma_start(out=o_fan[:], in_=sp_fan[:])
        nc.scalar.activation(sp_fae[0:1, 0:496], e_fae[0:1, 0:496],
                             AF.Ln, bias=1.0)
        nc.sync.dma_start(out=o_fae1[:], in_=sp_fae[:])
        nc.scalar.activation(sp_fae2[0:1, 0:512], e_fae2[0:1, 0:512],
                             AF.Ln, bias=1.0)
        nc.sync.dma_start(out=o_fae2[:], in_=sp_fae2[:])
        nc.scalar.activation(s_lse[:], s_sum[:], AF.Ln)
        nc.sync.dma_start(out=o_fs_lse[:], in_=s_lse[:])

    _split_multiwait(nc)
    return nc


def _pack_inputs(inputs):
    """Pack/transpose/convert the model weights into the DMA layouts."""
    g = {k: np.asarray(v) for k, v in inputs.items()}

    f32 = np.float32

    def packW(Wm):
        # [K, M] -> [128, (K//128) * M] (kh-major columns)
        K, M = Wm.shape
        return np.ascontiguousarray(
            Wm.reshape(K // 128, 128, M).transpose(1, 0, 2)).reshape(128, -1)

    def packB(b):
        return np.ascontiguousarray(
            np.asarray(b, f32).reshape(-1, 128).T)

    W_gcn = np.asarray(g["W_gcn"], f32)
    fs_W1 = np.asarray(g["fs_W1"], f32)
    fae_W1 = np.asarray(g["fae_W1"], f32)
    fae_W1_eff = np.concatenate([fae_W1[:G], 3.0 * fae_W1[G:]], axis=0)
    fan_W2 = np.asarray(g["fan_W2"], f32)
    fan_b2 = np.asarray(g["fan_b2"], f32)
    wd = (fan_W2[:, 0] - fan_W2[:, 1])[:, None]

    wpack = np.zeros((128, _WTOT), f32)

    def put(name, arr):
        o = _WOFF[name]
        wpack[:arr.shape[0], o:o + arr.shape[1]] = arr

    put("wfi1", packW(np.asarray(g["fi_W1"], f32)))
    put("wfi2", packW(np.asarray(g["fi_W2"], f32)))
    put("wgcn0", packW(W_gcn[0] / 3.0))
    put("wgcn1", packW(W_gcn[1] / 3.0))
    put("wg3", packW(3.0 * np.asarray(g["Wg"], f32)))
    put("wfs1", packW(3.0 * fs_W1))
    put("wfan1", packW(np.asarray(g["fan_W1"], f32)))
    put("wfae1", packW(fae_W1_eff))
    put("wfan2d", packW(wd))
    put("wfae2p", packW(np.asarray(g["fae_W2"], f32)))
    put("e128", np.eye(128, dtype=f32))

    ident2 = np.zeros((2, 32), f32)
    for k in range(2):
        ident2[k, k * 16:(k + 1) * 16] = 1.0
    put("ident2", ident2)
    ident4 = np.zeros((4, 64), f32)
    for k in range(4):
        ident4[k, k * 16:(k + 1) * 16] = 1.0
    put("ident4", ident4)
    put("bfi1T", np.asarray(g["fi_b1"], f32).reshape(2, 128))
    put("bfs1T", np.asarray(g["fs_b1"], f32).reshape(4, 128))
    # wfs2rep[p, kh*32+v] = fs_W2[kh*128+p, 0]
    fs_W2 = np.asarray(g["fs_W2"], f32)[:, 0]
    rep = np.repeat(fs_W2.reshape(4, 128).T[:, :, None], 32, axis=2)
    put("wfs2rep", rep.reshape(128, 128))

    bg = np.asarray(g["bg"], f32)
    bgs = np.stack([(v + 1) * bg for v in range(NN)], axis=1)  # [256, 32]
    bgsd = np.ascontiguousarray(
        bgs.reshape(2, 128, NN).transpose(1, 0, 2)).reshape(128, 64)

    bpack = np.zeros((128, 16), f32)
    bpack[:, 0:2] = packB(np.asarray(g["b_gcn"], f32)[0] / 3.0)
    bpack[:, 2:4] = packB(np.asarray(g["b_gcn"], f32)[1] / 3.0)
    bpack[:, 4:6] = packB(np.asarray(g["fi_b2"], f32) / 3.0)
    bpack[:, 6:8] = packB(g["fan_b1"])
    bpack[:, 8:12] = packB(g["fae_b1"])
    fae_b2 = float(np.asarray(g["fae_b2"], f32).reshape(-1)[0])
    bpack[0, 12:16] = [fan_b2[0] - fan_b2[1], fan_b2[1] - fan_b2[0],
                       fae_b2, -fae_b2]

    maskneg = np.full((32, 512), -30.0, f32)
    onehot = np.zeros((32, 512), f32)
    for v in range(1, NN):
        maskneg[v, 0:16 * v] = 0.0
        onehot[v, 16 * (v - 1):16 * v] = 1.0
    mpack = np.concatenate([maskneg, onehot], axis=1)

    shared = {
        "wpack": wpack.astype(bf16),
        "bpack": bpack,
        "bgsd": bgsd,
        "mpack": mpack,
    }

    hG0 = np.asarray(g["hG0"], f32)  # [B, G]
    in_maps = []
    for ci in range(NC):
        sl = hG0[ci * BL:(ci + 1) * BL]            # [16, 256]
        hg = np.ascontiguousarray(
            sl.T.reshape(2, 128, BL).transpose(1, 0, 2)).astype(bf16)
        m = dict(shared)
        m["hg0"] = np.ascontiguousarray(hg.reshape(128, 32))
        in_maps.append(m)
    return in_maps


def _assemble_loss(results):
    tot = 0.0
    for r in results:
        tot += float(r["o_fan"][0, :528].astype(np.float64).sum())
        tot += float(r["o_fae1"][0, :496].astype(np.float64).sum())
        tot += float(r["o_fae2"][0, :512].astype(np.float64).sum())
        tot += float((r["o_fs_lse"][1:32].astype(np.float64)
                      - r["o_fs_src"][1:32].astype(np.float64)).sum())
    return np.float32(tot / B)


def run(inputs, trace=False):
    _install_axon_hook()
    from concourse.bass_utils import run_bass_kernel_spmd
    if "prog" not in _BUILD_CACHE:
        _BUILD_CACHE["prog"] = _build_program()
    nc = _BUILD_CACHE["prog"]
    in_maps = _pack_inputs(inputs)
    res = run_bass_kernel_spmd(nc, in_maps, list(range(NC)), trace=trace)
    loss = _assemble_loss(res.results)
    return loss, res


def kernel(**inputs):
    assert int(inputs.get("N", NN)) == NN
    loss, _ = run(inputs, trace=False)
    return loss
